# revision 17
# baseline (speedup 1.0000x reference)
"""Trainium2 Bass kernel for nn_LocationSlayerRandom (SLAYER two-branch spiking net).

Contract: kernel(**inputs) takes the FULL unsharded inputs
  spike_input [32,156,1,1,2048] f32, W1 [512,156], W2 [20,512],
  Wl1 [512,2048], Wl2 [20,512], perm [156] i32
and returns the FULL output [32,20,1,1,2204] f32.

Strategy (8 cores, data-parallel over batch, 4 samples/core):

Branch 1 (per sample b):  u1 = W1 @ psp_t(si)   (psp linear => commutes)
  - psp_t(si): DVE tensor_tensor_scan over the 156-row input packed into
    5 x [128, T] regions (tails packed at 32b offsets + 4x 128-ch region),
    emitted as 1024-col chunks chained through the fp8 dst's last column
    so fc1 can start after the first two half-chunks instead of after two
    full 2048-scans.
  - fc1 fp8 DoubleRow (tail-region + main-region as the two k-subtiles),
    loop order b -> half -> m so each (b, half) finishes all four m-tiles
    and fc2 for that (half, b) can accumulate immediately afterwards.
  - thresholds fused from PSUM: m0..m2 on ACT as Sign(u1-10) (weights
    pre-scaled 0.5), m3 on DVE as (u1>=10)-0.5 (weights 1.0); the 0.5
    offset is folded into the host-side threshold T2[o,t].
  - fc2: per (half, ch) one PSUM accumulation group; each sample's four
    matmuls are emitted right after that sample's thresholds, interleaved
    with the next sample's fc1 (groups on distinct PSUM banks), so the
    kernel tail is only the last sample's chain. v-scan + compare + DMA
    drain per 512-col chunk.

Branch 2: a1 = Wl1 @ sipT (t-contraction, fp8 DR), c'-psp as a free-dim
  scan with reset pattern, threshold to fp8 {0,1}, fc2 in fp8 DR, scan,
  threshold. a1 m-blocks are emitted between fc1 samples as PE filler.

Numerics: fp8 matmuls with fp32 accumulate; exact-spike encodings; psp
scans keep f32 state, fp8 carries at the two intra-series chunk
boundaries add noise comparable to the fp8 element rounding. Margin to
the threshold 10 is ~7 on both branches, far above all rounding noise.

DMA: three queues (sync: siAB regions; gpsimd: sipT+Wl1T; scalar: other
weights) so the scan-critical and PE-critical inputs arrive in parallel.
Output is fp8 (exact 0/1), 20 rows per sample only.
"""

from contextlib import ExitStack

import numpy as np
import ml_dtypes

import concourse.bass as bass
import concourse.mybir as mybir
from concourse import bacc
from concourse import tile as tile_mod
from concourse.bass_utils import run_bass_kernel_spmd

F32 = mybir.dt.float32
BF16 = mybir.dt.bfloat16
FP8 = mybir.dt.float8e4
DR = mybir.MatmulPerfMode.DoubleRow
AL = mybir.AluOpType
AF = mybir.ActivationFunctionType
BF16_NP = ml_dtypes.bfloat16
FP8_NP = ml_dtypes.float8_e4m3

B, C_IN, T = 32, 156, 2048
HID, OUT_DIM = 512, 20
CP = 156                      # permuted taxel axis (branch-2 "time")
N_CORES = 8
B_PER = B // N_CORES          # 4 samples per core
ALPHA = float(np.exp(-1.0 / 10.0))
THETA = 10.0
NB2 = B_PER * CP              # 624, branch-2 packed free dim
KT = T // 128                 # 16 k-tiles over t
HC = 1024                     # scan chunk / threshold width
THR_DVE = {(2, 3), (3, 2), (3, 3)}  # (b, m) thresholds routed to DVE


def build_program(tc, outs, ins):
    nc = tc.nc
    out = outs["out"]

    with ExitStack() as ctx:
        consts = ctx.enter_context(tc.tile_pool(name="consts", bufs=1))
        work = ctx.enter_context(tc.tile_pool(name="work", bufs=1))
        sgp = ctx.enter_context(tc.tile_pool(name="sgp", bufs=16))
        mid = ctx.enter_context(tc.tile_pool(name="mid", bufs=4))
        psA = ctx.enter_context(tc.tile_pool(name="psA", bufs=3, space="PSUM"))
        psPA = ctx.enter_context(tc.tile_pool(name="psPA", bufs=1, space="PSUM"))

        # ---------------- constant patterns (gpsimd; SBUF only) ----------
        alpha_t = consts.tile([128, HC], F32, tag="alpha")
        nc.gpsimd.memset(alpha_t[:], ALPHA)
        alpha2_t = consts.tile([128, HC], F32, tag="alpha2")
        nc.gpsimd.memset(alpha2_t[:], ALPHA * ALPHA)
        pat624 = consts.tile([128, NB2], F32, tag="pat624")
        nc.gpsimd.memset(pat624[:], ALPHA)
        for j in range(B_PER):
            nc.gpsimd.memset(pat624[:, j * CP:j * CP + 1], 0.0)
        bias_m10 = consts.tile([128, 1], F32, tag="bm10")
        nc.gpsimd.memset(bias_m10[:], -THETA)
        act_warm = consts.tile([128, 1], F32, tag="actwarm")
        nc.scalar.activation(act_warm[:], bias_m10[:], AF.Sign,
                             bias=bias_m10[:])

        # ---------------- inputs (three parallel DMA queues) -------------
        # one sync-engine queue, strict priority order: the two chunk pairs
        # feeding the first scans, fc1 weights, then the rest by first use.
        siQ = consts.tile([128, 5 * HC], BF16, tag="siQ")
        siE = consts.tile([128, 5 * HC], FP8, tag="siE")
        w1c = consts.tile([128, 5 * 512], FP8, tag="w1c")
        sip = consts.tile([128, KT * NB2], FP8, tag="sip")
        wl1 = consts.tile([128, KT * HID], FP8, tag="wl1")
        w2p = consts.tile([128, 2 * 4 * 2 * 128], FP8, tag="w2p")
        t2_t = consts.tile([128, T], BF16, tag="t2")
        wl2 = consts.tile([128, 4 * 32], FP8, tag="wl2")
        HH = HC // 2
        nc.sync.dma_start(siQ[:, 0:HH], ins["siQ"][:, 0:HH])
        nc.sync.dma_start(siQ[:, HC:HC + HH], ins["siQ"][:, HC:HC + HH])
        nc.sync.dma_start(siE[:, 0:HH], ins["siE"][:, 0:HH])
        nc.sync.dma_start(siE[:, HC:HC + HH], ins["siE"][:, HC:HC + HH])
        nc.sync.dma_start(w1c[:], ins["W1c"][:])
        nc.sync.dma_start(siQ[:, HH:HC], ins["siQ"][:, HH:HC])
        nc.sync.dma_start(siQ[:, HC + HH:2 * HC], ins["siQ"][:, HC + HH:2 * HC])
        nc.sync.dma_start(siE[:, HH:HC], ins["siE"][:, HH:HC])
        nc.sync.dma_start(siE[:, HC + HH:2 * HC], ins["siE"][:, HC + HH:2 * HC])
        nc.sync.dma_start(siQ[:, 2 * HC:3 * HC], ins["siQ"][:, 2 * HC:3 * HC])
        nc.sync.dma_start(siE[:, 2 * HC:3 * HC], ins["siE"][:, 2 * HC:3 * HC])
        nc.sync.dma_start(sip[:], ins["sipT"][:])
        nc.sync.dma_start(wl1[:], ins["Wl1T"][:])
        nc.sync.dma_start(siQ[:, 3 * HC:4 * HC], ins["siQ"][:, 3 * HC:4 * HC])
        nc.sync.dma_start(siE[:, 3 * HC:4 * HC], ins["siE"][:, 3 * HC:4 * HC])
        nc.sync.dma_start(siQ[:, 4 * HC:5 * HC], ins["siQ"][:, 4 * HC:5 * HC])
        nc.sync.dma_start(siE[:, 4 * HC:5 * HC], ins["siE"][:, 4 * HC:5 * HC])
        nc.sync.dma_start(w2p[:], ins["W2pT"][:])
        for b in range(B_PER):
            nc.sync.dma_start(t2_t[32 * b:32 * b + OUT_DIM, :], ins["T2"][:])
        nc.sync.dma_start(wl2[:], ins["Wl2T"][:])

        # ---------------- branch-1 input psp (deg-2 scan + recovery) -----
        # z_k = y[2k+1] scans the host-paired q2 with ratio alpha^2; even
        # positions recover as y[2k] = alpha*z_{k-1} + x[2k] via one STT.
        # psABP pads 2 zero columns before each region so the STT's shifted
        # AP reads z_{-1} = 0.
        RP = T + 2
        psABP = work.tile([128, 5 * RP], FP8, tag="psABP")
        psABP3 = psABP[:].rearrange("p (r x) -> p r x", x=RP)
        for r in range(5):
            nc.gpsimd.memset(psABP[:, r * RP:r * RP + 2], 0.0)
        w1c3 = w1c[:].rearrange("p (r mj) -> p r mj", mj=512)

        def scan_chunk(r, h, nh=2):
            # z-scan chunk: q2 cols [h*512:(h+nh)*512] -> odd dst cols
            k0, k1 = h * (HC // 2), (h + nh) * (HC // 2)
            base = r * RP + 2
            init = (0.0 if h == 0 else
                    psABP[:, base + 2 * k0 - 1:base + 2 * k0])
            nc.vector.tensor_tensor_scan(
                psABP3[:, r, 2 + 2 * k0 + 1:2 + 2 * k1:2],
                alpha2_t[:, 0:k1 - k0], siQ[:, r * HC + k0:r * HC + k1],
                init, AL.mult, AL.add)

        def stt_chunk(r, h, nh=2):
            k0, k1 = h * (HC // 2), (h + nh) * (HC // 2)
            nc.vector.scalar_tensor_tensor(
                psABP3[:, r, 2 + 2 * k0:2 + 2 * k1:2],
                psABP3[:, r, 1 + 2 * k0:1 + 2 * k1:2], ALPHA,
                siE[:, r * HC + k0:r * HC + k1], AL.mult, AL.add)


        # ---------------- branch 2 A1 block emitter ----------------------
        wl1_3d = wl1[:].rearrange("p (k o) -> p k o", o=HID)
        sip_3d = sip[:].rearrange("p (k c) -> p k c", c=NB2)
        l1 = work.tile([128, 4 * NB2], FP8, tag="l1")
        l1_3d = l1[:].rearrange("p (m c) -> p m c", c=NB2)

        def a1_block(m):
            pa = psPA.tile([128, 1024], F32, tag="psPA", name=f"pa{m}")
            a1 = pa[:, :NB2]
            msl = slice(m * 128, (m + 1) * 128)
            for ki in range(KT // 2):
                st, sp = (ki == 0), (ki == KT // 2 - 1)
                lhs = wl1_3d[:, 2 * ki:2 * ki + 2, msl]
                nc.tensor.matmul(a1[:, 0:512], lhs,
                                 sip_3d[:, 2 * ki:2 * ki + 2, 0:512],
                                 start=st, stop=sp, perf_mode=DR)
                nc.tensor.matmul(a1[:, 512:NB2], lhs,
                                 sip_3d[:, 2 * ki:2 * ki + 2, 512:NB2],
                                 start=st, stop=sp, perf_mode=DR)
            u = mid.tile([128, NB2], BF16, tag="ul1", name=f"ul1{m}")
            nc.vector.tensor_tensor_scan(u[:], pat624[:], a1, 0.0,
                                         AL.mult, AL.add)
            # l1 in fp8 {0,1} (exact) for the fp8 DR fc2
            nc.vector.tensor_scalar(l1_3d[:, m, :], u[:], THETA, None,
                                    AL.is_ge)

        # ---------------- branch 1 fc1 + fc2 interleave ------------------
        # sg pair tiles per (b, kp): [128, 2*T] fp8, layout [p, (s t)] so
        # fc2 pairs the two m-tiles of kp as DoubleRow k-subtiles.
        sgt = {}
        for b in range(B_PER):
            for kp in range(2):
                sgt[(b, kp)] = sgp.tile([128, 2 * T], FP8, tag="sg",
                                        name=f"sg{b}{kp}")
        puh = {}
        w2blk4 = w2p[:].rearrange("p (kp b s j) -> p kp b s j",
                                  b=4, s=2, j=128)

        def fc1_b(b):
            if b == 0:
                order = [(m, h) for h in range(2) for m in range(4)]
            else:
                order = [(m, h) for m in range(4) for h in range(2)]
            for m, half in order:
                s_t = sgt[(b, m // 2)][:, (m % 2) * T:(m % 2) * T + T]
                msl = slice(m * 128, (m + 1) * 128)
                lhsT = w1c3[:, b:5:(4 - b), msl]
                if True:
                    pu = psA.tile([128, 1024], F32, tag="psA")
                    for ch in range(2):
                        c0 = 2 + half * 1024 + ch * 512
                        nc.tensor.matmul(pu[:, ch * 512:(ch + 1) * 512],
                                         lhsT,
                                         psABP3[:, 0:b + 2:b + 1, c0:c0 + 512],
                                         start=True, stop=True, perf_mode=DR)
                    hsl = slice(half * 1024, (half + 1) * 1024)
                    if (b, m) in THR_DVE:
                        nc.vector.tensor_scalar(s_t[:, hsl], pu[:], THETA,
                                                0.5, AL.is_ge, AL.subtract)
                    else:
                        nc.scalar.activation(s_t[:, hsl], pu[:], AF.Sign,
                                             bias=bias_m10[:])

        def fc2_hf(hf):
            puh[hf] = psA.tile([128, 1024], F32, tag="psA", name=f"pu2{hf}")
            for ch in range(2):
                c0 = hf * 1024 + ch * 512
                for kp in range(2):
                    for b in range(B_PER):
                        rhs3 = sgt[(b, kp)][:].rearrange(
                            "p (s t) -> p s t", t=T)
                        nc.tensor.matmul(puh[hf][:, ch * 512:(ch + 1) * 512],
                                         w2blk4[:, kp, b, :, :],
                                         rhs3[:, :, c0:c0 + 512],
                                         start=(kp == 0 and b == 0),
                                         stop=(kp == 1 and b == 3),
                                         perf_mode=DR)

        # PE stream: a1 blocks slot into fc1's scan-wait gaps; their DVE
        # scans are interleaved between input chunks (emitted inside
        # a1_block) so pa PSUM tiles free quickly.
        scan_chunk(0, 0, 1)
        scan_chunk(1, 0, 1)
        stt_chunk(0, 0, 1)
        stt_chunk(1, 0, 1)
        scan_chunk(0, 1, 1)
        scan_chunk(1, 1, 1)
        stt_chunk(0, 1, 1)
        stt_chunk(1, 1, 1)
        fc1_b(0)
        scan_chunk(2, 0)
        stt_chunk(2, 0)
        a1_block(0)          # also emits its DVE scan + l1 compare here
        fc1_b(1)
        scan_chunk(3, 0)
        stt_chunk(3, 0)
        a1_block(1)
        fc1_b(2)
        scan_chunk(4, 0)
        stt_chunk(4, 0)
        a1_block(2)
        fc1_b(3)
        a1_block(3)
        fc2_hf(0)

        # ---------------- branch 2 fc2 (fp8 DR) + psp + out --------------
        wl2_3d = wl2[:].rearrange("p (k o) -> p k o", o=32)
        pl2full = psA.tile([128, 1024], F32, tag="psA", name="pl2")
        pl2 = pl2full[:32, :NB2]
        for kp in range(2):
            st, sp = (kp == 0), (kp == 1)
            lhsT = wl2_3d[:, 2 * kp:2 * kp + 2, :]
            nc.tensor.matmul(pl2[:, 0:512], lhsT,
                             l1_3d[:, 2 * kp:2 * kp + 2, 0:512],
                             start=st, stop=sp, perf_mode=DR)
            nc.tensor.matmul(pl2[:, 512:NB2], lhsT,
                             l1_3d[:, 2 * kp:2 * kp + 2, 512:NB2],
                             start=st, stop=sp, perf_mode=DR)
        fc2_hf(1)
        ul2 = mid.tile([128, NB2], F32, tag="ul2")
        nc.vector.tensor_tensor_scan(ul2[:32], pat624[:32], pl2,
                                     0.0, AL.mult, AL.add)
        o2 = mid.tile([128, NB2], BF16, tag="o2")
        nc.vector.tensor_scalar(o2[:OUT_DIM], ul2[:OUT_DIM], THETA, None,
                                AL.is_ge)
        nc.sync.dma_start(
            out[:, :OUT_DIM, T:T + CP].rearrange("b o c -> o b c"),
            o2[:OUT_DIM, :].rearrange("o (b c) -> o b c", c=CP))

        # ---------------- branch 1 v-scan + compare + out ----------------
        vs = work.tile([128, T], BF16, tag="vs")
        o1 = work.tile([128, T], BF16, tag="o1")

        vcarry = work.tile([128, 1], F32, tag="vcarry")
        vmid = work.tile([128, 1], F32, tag="vmid")
        prev_init = 0.0
        for hf in range(2):
            q0 = hf * 1024
            nc.vector.tensor_tensor_scan(vs[:, q0:q0 + 512],
                                         alpha_t[:, 0:512],
                                         puh[hf][:, 0:512],
                                         prev_init, AL.mult, AL.add)
            nc.vector.tensor_scalar(vmid[:], vs[:, q0 + 511:q0 + 512],
                                    1.0, None, AL.mult)
            nc.vector.tensor_tensor_scan(vs[:, q0 + 512:q0 + 1024],
                                         alpha_t[:, 0:512],
                                         puh[hf][:, 512:1024],
                                         vmid[:, 0:1], AL.mult, AL.add)
            if hf == 0:
                nc.vector.tensor_scalar(vcarry[:], vs[:, 1023:1024],
                                        1.0, None, AL.mult)
                prev_init = vcarry[:, 0:1]
            for q in range(2):
                qsl = slice(q0 + q * 512, q0 + (q + 1) * 512)
                nc.vector.tensor_tensor(o1[:, qsl], vs[:, qsl], t2_t[:, qsl],
                                        AL.is_ge)
                nc.sync.dma_start(
                    out[:, :, qsl].rearrange("b j t -> (b j) t"),
                    o1[:, qsl])


# ======================= host-side preparation =======================

def prep_core_inputs(si, sip, core):
    """Per-core data tensors, pre-packed into single-DMA SBUF layouts.
    si/sip are [32,156,2048] f32 (sip already perm-gathered)."""
    sl = si[core * B_PER:(core + 1) * B_PER]          # [4,156,2048]
    # region layout: 0 = packed tails (rows 32b..32b+27), 1+b = 128-ch
    siAB = np.zeros((128, 5, T), dtype=np.float32)
    for b in range(B_PER):
        siAB[:, 1 + b, :] = sl[b, :128, :]
        siAB[32 * b:32 * b + (C_IN - 128), 0, :] = sl[b, 128:C_IN, :]
    # deg-2 pairing: q2[k] = a*x[2k] + x[2k+1] (bf16), xe[k] = x[2k] (fp8)
    siQ = (ALPHA * siAB[:, :, 0::2] + siAB[:, :, 1::2]).reshape(
        128, 5 * (T // 2)).astype(BF16_NP)
    siE = siAB[:, :, 0::2].reshape(128, 5 * (T // 2)).astype(FP8_NP)
    sp = sip[core * B_PER:(core + 1) * B_PER]         # [4,156,2048]
    # sipT [128, KT*NB2]: [p, k*NB2 + b*CP + c'] = sip[b, c', 128k+p]
    sipT = np.ascontiguousarray(
        sp.transpose(2, 0, 1).reshape(KT, 128, NB2)
        .transpose(1, 0, 2).reshape(128, KT * NB2)
    ).astype(FP8_NP)
    return {"siQ": siQ, "siE": siE, "sipT": sipT}


def prep_shared_inputs(W1, W2, Wl1, Wl2):
    """Weight layouts + threshold tensor, shared by all cores."""
    # W1c [128, 5*512] fp8: region b = tail weights for sample b at rows
    # 32b..32b+27 (zero elsewhere); region 4 = W1^T[0:128].
    w1t = np.zeros((160, HID), dtype=np.float32)
    w1t[:C_IN] = W1.T
    W1c = np.zeros((128, 5 * 512), dtype=FP8_NP)
    W1c[:, 4 * 512:5 * 512] = w1t[:128]
    for b in range(B_PER):
        W1c[32 * b:32 * b + (C_IN - 128), b * 512:(b + 1) * 512] = \
            w1t[128:C_IN]

    # fc2 block weights for DoubleRow: per (kp, b, s) a [128, 128] block,
    # zero except cols 32b..32b+20 = k_scale[m]*W2_m^T (m = 2kp+s). Per-m
    # scale matches the sg encoding: ACT Sign (+-1, m0..m2) -> 0.5x, DVE
    # (+-.5, m3) -> 1.0x. Layout [128, 2*4*2*128].
    # scale per (b, m): DVE is_ge-0.5 encoding (+-.5) -> 1.0x, ACT Sign
    # (+-1) -> 0.5x; routing must match THR_DVE in the device program.
    w2t = W2.T.astype(np.float32)                     # [512, 20]
    W2pT = np.zeros((128, 2 * 4 * 2 * 128), dtype=FP8_NP)
    for kp in range(2):
        for s in range(2):
            m = 2 * kp + s
            for b in range(B_PER):
                sc = 1.0 if (b, m) in THR_DVE else 0.5
                blk = (sc * w2t[m * 128:(m + 1) * 128]).astype(FP8_NP)
                base = ((kp * 4 + b) * 2 + s) * 128 + 32 * b
                W2pT[:, base:base + OUT_DIM] = blk
    # effective (device) W2 after fp8 rounding, unscaled (b=0 blocks)
    w2_eff = np.empty((HID, OUT_DIM), dtype=np.float32)
    for m in range(4):
        kp, s = m // 2, m % 2
        sc = 1.0 if (0, m) in THR_DVE else 0.5
        base = ((kp * 4 + 0) * 2 + s) * 128 + 0
        w2_eff[m * 128:(m + 1) * 128] = (
            W2pT[:, base:base + OUT_DIM].astype(np.float32) / sc)
    r2 = w2_eff.sum(axis=0)                           # [20]
    g = (1.0 - ALPHA ** (np.arange(T, dtype=np.float64) + 1)) / (1.0 - ALPHA)
    T2 = (THETA - 0.5 * np.outer(r2, g)).astype(np.float32)   # [20, T]
    T2 = T2.astype(BF16_NP)

    # Wl1T [128, KT*HID]: [p, k*HID+o] = Wl1[o, 128k+p]
    Wl1T = np.ascontiguousarray(
        Wl1.T.reshape(KT, 128, HID).transpose(1, 0, 2).reshape(128, KT * HID)
    ).astype(FP8_NP)
    # Wl2T [128, 4*32] fp8 (o padded to 32): [p, k*32+o] = Wl2[o, 128k+p]
    wl2t = Wl2.T.reshape(4, 128, OUT_DIM)
    Wl2T = np.zeros((4, 128, 32), dtype=np.float32)
    Wl2T[:, :, :OUT_DIM] = wl2t
    Wl2T = np.ascontiguousarray(
        Wl2T.transpose(1, 0, 2).reshape(128, 4 * 32)).astype(FP8_NP)
    return {"W1c": W1c, "W2pT": W2pT, "Wl1T": Wl1T,
            "Wl2T": Wl2T, "T2": T2}


def make_in_maps(spike_input, W1, W2, Wl1, Wl2, perm):
    si = np.asarray(spike_input, dtype=np.float32).reshape(B, C_IN, T)
    perm = np.asarray(perm).astype(np.int64)
    sip = si[:, perm, :]                              # perm-gather (layout only)
    shared = prep_shared_inputs(np.asarray(W1, np.float32),
                                np.asarray(W2, np.float32),
                                np.asarray(Wl1, np.float32),
                                np.asarray(Wl2, np.float32))
    in_maps = []
    for core in range(N_CORES):
        m = dict(shared)
        m.update(prep_core_inputs(si, sip, core))
        in_maps.append(m)
    return in_maps


_IN_SPECS = {
    "siQ": ((128, 5 * (T // 2)), BF16),
    "siE": ((128, 5 * (T // 2)), FP8),
    "sipT": ((128, KT * NB2), FP8),
    "W1c": ((128, 5 * 512), FP8),
    "W2pT": ((128, 2 * 4 * 2 * 128), FP8),
    "Wl1T": ((128, KT * HID), FP8),
    "Wl2T": ((128, 4 * 32), FP8),
    "T2": ((OUT_DIM, T), BF16),
}


def build_bass():
    nc = bacc.Bacc("TRN2", target_bir_lowering=False, debug=False)
    ins = {}
    for name, (shape, dt) in _IN_SPECS.items():
        h = nc.dram_tensor(name, list(shape), dt, kind="ExternalInput")
        ins[name] = h[:]
    out_h = nc.dram_tensor("out", [B_PER, 32, T + CP], BF16,
                           kind="ExternalOutput")
    outs = {"out": out_h[:]}
    with tile_mod.TileContext(nc) as tc:
        build_program(tc, outs, ins)
    nc.compile()
    return nc


_NC_CACHE = None


def run(inputs, trace=False, **kw):
    """Run on the 8 NeuronCores; returns (full_output, BassKernelResults)."""
    global _NC_CACHE
    if _NC_CACHE is None:
        _NC_CACHE = build_bass()
    nc = _NC_CACHE
    in_maps = make_in_maps(**inputs)
    res = run_bass_kernel_spmd(nc, in_maps, core_ids=list(range(N_CORES)),
                               trace=trace, **kw)
    parts = [res.results[c]["out"][:, :OUT_DIM, :] for c in range(N_CORES)]
    full = np.concatenate(parts, axis=0).reshape(B, OUT_DIM, 1, 1, T + CP)
    return np.ascontiguousarray(full.astype(np.float32)), res


def kernel(**inputs):
    out, _ = run(inputs)
    return out


# revision 18
# speedup vs baseline: 1.1598x; 1.1598x over previous
"""Trainium2 Bass kernel for nn_LocationSlayerRandom (SLAYER two-branch spiking net).

Contract: kernel(**inputs) takes the FULL unsharded inputs
  spike_input [32,156,1,1,2048] f32, W1 [512,156], W2 [20,512],
  Wl1 [512,2048], Wl2 [20,512], perm [156] i32
and returns the FULL output [32,20,1,1,2204] f32.

Strategy (8 cores, data-parallel over batch, 4 samples/core):

Branch 1 (per sample b):  u1 = W1 @ psp_t(si)   (psp linear => commutes)
  - psp_t(si): DVE tensor_tensor_scan over the 156-row input packed into
    5 x [128, T] regions (tails packed at 32b offsets + 4x 128-ch region),
    emitted as 1024-col chunks chained through the fp8 dst's last column
    so fc1 can start after the first two half-chunks instead of after two
    full 2048-scans.
  - fc1 fp8 DoubleRow (tail-region + main-region as the two k-subtiles),
    loop order b -> half -> m so each (b, half) finishes all four m-tiles
    and fc2 for that (half, b) can accumulate immediately afterwards.
  - thresholds fused from PSUM: m0..m2 on ACT as Sign(u1-10) (weights
    pre-scaled 0.5), m3 on DVE as (u1>=10)-0.5 (weights 1.0); the 0.5
    offset is folded into the host-side threshold T2[o,t].
  - fc2: per (half, ch) one PSUM accumulation group; each sample's four
    matmuls are emitted right after that sample's thresholds, interleaved
    with the next sample's fc1 (groups on distinct PSUM banks), so the
    kernel tail is only the last sample's chain. v-scan + compare + DMA
    drain per 512-col chunk.

Branch 2: a1 = Wl1 @ sipT (t-contraction, fp8 DR), c'-psp as a free-dim
  scan with reset pattern, threshold to fp8 {0,1}, fc2 in fp8 DR, scan,
  threshold. a1 m-blocks are emitted between fc1 samples as PE filler.

Numerics: fp8 matmuls with fp32 accumulate; exact-spike encodings; psp
scans keep f32 state, fp8 carries at the two intra-series chunk
boundaries add noise comparable to the fp8 element rounding. Margin to
the threshold 10 is ~7 on both branches, far above all rounding noise.

DMA: three queues (sync: siAB regions; gpsimd: sipT+Wl1T; scalar: other
weights) so the scan-critical and PE-critical inputs arrive in parallel.
Output is fp8 (exact 0/1), 20 rows per sample only.
"""

from contextlib import ExitStack

import numpy as np
import ml_dtypes

import concourse.bass as bass
import concourse.mybir as mybir
from concourse import bacc
from concourse import tile as tile_mod
from concourse.bass_utils import run_bass_kernel_spmd

F32 = mybir.dt.float32
BF16 = mybir.dt.bfloat16
FP8 = mybir.dt.float8e4
DR = mybir.MatmulPerfMode.DoubleRow
AL = mybir.AluOpType
AF = mybir.ActivationFunctionType
BF16_NP = ml_dtypes.bfloat16
FP8_NP = ml_dtypes.float8_e4m3

B, C_IN, T = 32, 156, 2048
HID, OUT_DIM = 512, 20
CP = 156                      # permuted taxel axis (branch-2 "time")
N_CORES = 8
B_PER = B // N_CORES          # 4 samples per core
ALPHA = float(np.exp(-1.0 / 10.0))
THETA = 10.0
NB2 = B_PER * CP              # 624, branch-2 packed free dim
KT = T // 128                 # 16 k-tiles over t
HC = 1024                     # scan chunk / threshold width
THR_DVE = {(2, 2), (2, 3), (3, 2), (3, 3)}  # (b, m) thresholds routed to DVE


def build_program(tc, outs, ins):
    nc = tc.nc
    out = outs["out"]

    with ExitStack() as ctx:
        consts = ctx.enter_context(tc.tile_pool(name="consts", bufs=1))
        work = ctx.enter_context(tc.tile_pool(name="work", bufs=1))
        sgp = ctx.enter_context(tc.tile_pool(name="sgp", bufs=16))
        mid = ctx.enter_context(tc.tile_pool(name="mid", bufs=4))
        psA = ctx.enter_context(tc.tile_pool(name="psA", bufs=3, space="PSUM"))
        psPA = ctx.enter_context(tc.tile_pool(name="psPA", bufs=1, space="PSUM"))

        # ---------------- constant patterns (gpsimd; SBUF only) ----------
        alpha_t = consts.tile([128, HC], F32, tag="alpha")
        nc.gpsimd.memset(alpha_t[:], ALPHA)
        alpha2_t = consts.tile([128, HC], F32, tag="alpha2")
        nc.gpsimd.memset(alpha2_t[:], ALPHA * ALPHA)
        pat624 = consts.tile([128, NB2], F32, tag="pat624")
        nc.gpsimd.memset(pat624[:], ALPHA)
        for j in range(B_PER):
            nc.gpsimd.memset(pat624[:, j * CP:j * CP + 1], 0.0)
        bias_m10 = consts.tile([128, 1], F32, tag="bm10")
        nc.gpsimd.memset(bias_m10[:], -THETA)
        act_warm = consts.tile([128, 1], F32, tag="actwarm")
        nc.scalar.activation(act_warm[:], bias_m10[:], AF.Sign,
                             bias=bias_m10[:])

        # ---------------- inputs (three parallel DMA queues) -------------
        # one sync-engine queue, strict priority order: the two chunk pairs
        # feeding the first scans, fc1 weights, then the rest by first use.
        siQ = consts.tile([128, 5 * HC], BF16, tag="siQ")
        siE = consts.tile([128, 5 * HC], FP8, tag="siE")
        w1c = consts.tile([128, 5 * 512], FP8, tag="w1c")
        sip = consts.tile([128, KT * NB2], FP8, tag="sip")
        wl1 = consts.tile([128, KT * HID], FP8, tag="wl1")
        w2p = consts.tile([128, 2 * 4 * 2 * 128], FP8, tag="w2p")
        t2_t = consts.tile([128, T], BF16, tag="t2")
        wl2 = consts.tile([128, 4 * 32], FP8, tag="wl2")
        HH = HC // 2
        nc.sync.dma_start(siQ[:, 0:HH], ins["siQ"][:, 0:HH])
        nc.sync.dma_start(siQ[:, HC:HC + HH], ins["siQ"][:, HC:HC + HH])
        nc.sync.dma_start(siE[:, 0:HH], ins["siE"][:, 0:HH])
        nc.sync.dma_start(siE[:, HC:HC + HH], ins["siE"][:, HC:HC + HH])
        nc.sync.dma_start(w1c[:], ins["W1c"][:])
        nc.sync.dma_start(siQ[:, HH:HC], ins["siQ"][:, HH:HC])
        nc.sync.dma_start(siQ[:, HC + HH:2 * HC], ins["siQ"][:, HC + HH:2 * HC])
        nc.sync.dma_start(siE[:, HH:HC], ins["siE"][:, HH:HC])
        nc.sync.dma_start(siE[:, HC + HH:2 * HC], ins["siE"][:, HC + HH:2 * HC])
        nc.sync.dma_start(siQ[:, 2 * HC:3 * HC], ins["siQ"][:, 2 * HC:3 * HC])
        nc.sync.dma_start(siE[:, 2 * HC:3 * HC], ins["siE"][:, 2 * HC:3 * HC])
        nc.sync.dma_start(sip[:], ins["sipT"][:])
        nc.sync.dma_start(wl1[:], ins["Wl1T"][:])
        nc.sync.dma_start(siQ[:, 3 * HC:4 * HC], ins["siQ"][:, 3 * HC:4 * HC])
        nc.sync.dma_start(siE[:, 3 * HC:4 * HC], ins["siE"][:, 3 * HC:4 * HC])
        nc.sync.dma_start(siQ[:, 4 * HC:5 * HC], ins["siQ"][:, 4 * HC:5 * HC])
        nc.sync.dma_start(siE[:, 4 * HC:5 * HC], ins["siE"][:, 4 * HC:5 * HC])
        nc.sync.dma_start(w2p[:], ins["W2pT"][:])
        for b in range(B_PER):
            nc.sync.dma_start(t2_t[32 * b:32 * b + OUT_DIM, :], ins["T2"][:])
        nc.sync.dma_start(wl2[:], ins["Wl2T"][:])

        # ---------------- branch-1 input psp (deg-2 scan + recovery) -----
        # z_k = y[2k+1] scans the host-paired q2 with ratio alpha^2; even
        # positions recover as y[2k] = alpha*z_{k-1} + x[2k] via one STT.
        # psABP pads 2 zero columns before each region so the STT's shifted
        # AP reads z_{-1} = 0.
        RP = T + 2
        psABP = work.tile([128, 5 * RP], FP8, tag="psABP")
        psABP3 = psABP[:].rearrange("p (r x) -> p r x", x=RP)
        for r in range(5):
            nc.gpsimd.memset(psABP[:, r * RP:r * RP + 2], 0.0)
        w1c3 = w1c[:].rearrange("p (r mj) -> p r mj", mj=512)

        def scan_chunk(r, h, nh=2):
            # z-scan chunk: q2 cols [h*512:(h+nh)*512] -> odd dst cols
            k0, k1 = h * (HC // 2), (h + nh) * (HC // 2)
            base = r * RP + 2
            init = (0.0 if h == 0 else
                    psABP[:, base + 2 * k0 - 1:base + 2 * k0])
            nc.vector.tensor_tensor_scan(
                psABP3[:, r, 2 + 2 * k0 + 1:2 + 2 * k1:2],
                alpha2_t[:, 0:k1 - k0], siQ[:, r * HC + k0:r * HC + k1],
                init, AL.mult, AL.add)

        def stt_chunk(r, h, nh=2):
            k0, k1 = h * (HC // 2), (h + nh) * (HC // 2)
            nc.vector.scalar_tensor_tensor(
                psABP3[:, r, 2 + 2 * k0:2 + 2 * k1:2],
                psABP3[:, r, 1 + 2 * k0:1 + 2 * k1:2], ALPHA,
                siE[:, r * HC + k0:r * HC + k1], AL.mult, AL.add)


        # ---------------- branch 2 A1 block emitter ----------------------
        wl1_3d = wl1[:].rearrange("p (k o) -> p k o", o=HID)
        sip_3d = sip[:].rearrange("p (k c) -> p k c", c=NB2)
        l1 = work.tile([128, 4 * NB2], FP8, tag="l1")
        l1_3d = l1[:].rearrange("p (m c) -> p m c", c=NB2)

        def a1_block(m):
            pa = psPA.tile([128, 1024], F32, tag="psPA", name=f"pa{m}")
            a1 = pa[:, :NB2]
            msl = slice(m * 128, (m + 1) * 128)
            for ki in range(KT // 2):
                st, sp = (ki == 0), (ki == KT // 2 - 1)
                lhs = wl1_3d[:, 2 * ki:2 * ki + 2, msl]
                nc.tensor.matmul(a1[:, 0:512], lhs,
                                 sip_3d[:, 2 * ki:2 * ki + 2, 0:512],
                                 start=st, stop=sp, perf_mode=DR)
                nc.tensor.matmul(a1[:, 512:NB2], lhs,
                                 sip_3d[:, 2 * ki:2 * ki + 2, 512:NB2],
                                 start=st, stop=sp, perf_mode=DR)
            u = mid.tile([128, NB2], BF16, tag="ul1", name=f"ul1{m}")
            nc.vector.tensor_tensor_scan(u[:], pat624[:], a1, 0.0,
                                         AL.mult, AL.add)
            # l1 in fp8 {0,1} (exact) for the fp8 DR fc2
            nc.vector.tensor_scalar(l1_3d[:, m, :], u[:], THETA, None,
                                    AL.is_ge)

        # ---------------- branch 1 fc1 + fc2 interleave ------------------
        # sg pair tiles per (b, kp): [128, 2*T] fp8, layout [p, (s t)] so
        # fc2 pairs the two m-tiles of kp as DoubleRow k-subtiles.
        sgt = {}
        for b in range(B_PER):
            for kp in range(2):
                sgt[(b, kp)] = sgp.tile([128, 2 * T], FP8, tag="sg",
                                        name=f"sg{b}{kp}")
        puh = {}
        w2blk4 = w2p[:].rearrange("p (kp b s j) -> p kp b s j",
                                  b=4, s=2, j=128)

        def fc1_b(b):
            for m, half in [(m, h) for m in range(4) for h in range(2)]:
                s_t = sgt[(b, m // 2)][:, (m % 2) * T:(m % 2) * T + T]
                msl = slice(m * 128, (m + 1) * 128)
                lhsT = w1c3[:, b:5:(4 - b), msl]
                if True:
                    pu = psA.tile([128, 1024], F32, tag="psA")
                    for ch in range(2):
                        c0 = 2 + half * 1024 + ch * 512
                        nc.tensor.matmul(pu[:, ch * 512:(ch + 1) * 512],
                                         lhsT,
                                         psABP3[:, 0:b + 2:b + 1, c0:c0 + 512],
                                         start=True, stop=True, perf_mode=DR)
                    hsl = slice(half * 1024, (half + 1) * 1024)
                    if (b, m) in THR_DVE:
                        nc.vector.tensor_scalar(s_t[:, hsl], pu[:], THETA,
                                                0.5, AL.is_ge, AL.subtract)
                    else:
                        nc.scalar.activation(s_t[:, hsl], pu[:], AF.Sign,
                                             bias=bias_m10[:])

        def fc2_hf(hf):
            puh[hf] = psA.tile([128, 1024], F32, tag="psA", name=f"pu2{hf}")
            for ch in range(2):
                c0 = hf * 1024 + ch * 512
                for kp in range(2):
                    for b in range(B_PER):
                        rhs3 = sgt[(b, kp)][:].rearrange(
                            "p (s t) -> p s t", t=T)
                        nc.tensor.matmul(puh[hf][:, ch * 512:(ch + 1) * 512],
                                         w2blk4[:, kp, b, :, :],
                                         rhs3[:, :, c0:c0 + 512],
                                         start=(kp == 0 and b == 0),
                                         stop=(kp == 1 and b == 3),
                                         perf_mode=DR)

        # PE stream: a1 blocks slot into fc1's scan-wait gaps; their DVE
        # scans are interleaved between input chunks (emitted inside
        # a1_block) so pa PSUM tiles free quickly.
        scan_chunk(0, 0, 1)
        scan_chunk(1, 0, 1)
        stt_chunk(0, 0, 1)
        stt_chunk(1, 0, 1)
        scan_chunk(0, 1, 1)
        scan_chunk(1, 1, 1)
        stt_chunk(0, 1, 1)
        stt_chunk(1, 1, 1)
        fc1_b(0)
        scan_chunk(2, 0)
        stt_chunk(2, 0)
        a1_block(0)          # also emits its DVE scan + l1 compare here
        fc1_b(1)
        scan_chunk(3, 0)
        stt_chunk(3, 0)
        a1_block(1)
        fc1_b(2)
        scan_chunk(4, 0)
        stt_chunk(4, 0)
        a1_block(2)
        fc1_b(3)
        a1_block(3)
        fc2_hf(0)

        # ---------------- branch 2 fc2 (fp8 DR) + psp + out --------------
        wl2_3d = wl2[:].rearrange("p (k o) -> p k o", o=32)
        pl2full = psA.tile([128, 1024], F32, tag="psA", name="pl2")
        pl2 = pl2full[:32, :NB2]
        for kp in range(2):
            st, sp = (kp == 0), (kp == 1)
            lhsT = wl2_3d[:, 2 * kp:2 * kp + 2, :]
            nc.tensor.matmul(pl2[:, 0:512], lhsT,
                             l1_3d[:, 2 * kp:2 * kp + 2, 0:512],
                             start=st, stop=sp, perf_mode=DR)
            nc.tensor.matmul(pl2[:, 512:NB2], lhsT,
                             l1_3d[:, 2 * kp:2 * kp + 2, 512:NB2],
                             start=st, stop=sp, perf_mode=DR)
        fc2_hf(1)
        ul2 = mid.tile([128, NB2], F32, tag="ul2")
        nc.vector.tensor_tensor_scan(ul2[:32], pat624[:32], pl2,
                                     0.0, AL.mult, AL.add)
        o2 = mid.tile([128, NB2], BF16, tag="o2")
        nc.vector.tensor_scalar(o2[:OUT_DIM], ul2[:OUT_DIM], THETA, None,
                                AL.is_ge)
        nc.sync.dma_start(
            out[:, :OUT_DIM, T:T + CP].rearrange("b o c -> o b c"),
            o2[:OUT_DIM, :].rearrange("o (b c) -> o b c", c=CP))

        # ---------------- branch 1 v-scan + compare + out ----------------
        vs = work.tile([128, T], BF16, tag="vs")
        o1 = work.tile([128, T], BF16, tag="o1")

        vcarry = work.tile([128, 1], F32, tag="vcarry")
        vmid = work.tile([128, 1], F32, tag="vmid")
        prev_init = 0.0
        for hf in range(2):
            q0 = hf * 1024
            nc.vector.tensor_tensor_scan(vs[:, q0:q0 + 512],
                                         alpha_t[:, 0:512],
                                         puh[hf][:, 0:512],
                                         prev_init, AL.mult, AL.add)
            nc.vector.tensor_scalar(vmid[:], vs[:, q0 + 511:q0 + 512],
                                    1.0, None, AL.mult)
            nc.vector.tensor_tensor_scan(vs[:, q0 + 512:q0 + 1024],
                                         alpha_t[:, 0:512],
                                         puh[hf][:, 512:1024],
                                         vmid[:, 0:1], AL.mult, AL.add)
            if hf == 0:
                nc.vector.tensor_scalar(vcarry[:], vs[:, 1023:1024],
                                        1.0, None, AL.mult)
                prev_init = vcarry[:, 0:1]
            for q in range(2):
                qsl = slice(q0 + q * 512, q0 + (q + 1) * 512)
                nc.vector.tensor_tensor(o1[:, qsl], vs[:, qsl], t2_t[:, qsl],
                                        AL.is_ge)
                nc.sync.dma_start(
                    out[:, :, qsl].rearrange("b j t -> (b j) t"),
                    o1[:, qsl])


# ======================= host-side preparation =======================

def prep_core_inputs(si, sip, core):
    """Per-core data tensors, pre-packed into single-DMA SBUF layouts.
    si/sip are [32,156,2048] f32 (sip already perm-gathered)."""
    sl = si[core * B_PER:(core + 1) * B_PER]          # [4,156,2048]
    # region layout: 0 = packed tails (rows 32b..32b+27), 1+b = 128-ch
    siAB = np.zeros((128, 5, T), dtype=np.float32)
    for b in range(B_PER):
        siAB[:, 1 + b, :] = sl[b, :128, :]
        siAB[32 * b:32 * b + (C_IN - 128), 0, :] = sl[b, 128:C_IN, :]
    # deg-2 pairing: q2[k] = a*x[2k] + x[2k+1] (bf16), xe[k] = x[2k] (fp8)
    siQ = (ALPHA * siAB[:, :, 0::2] + siAB[:, :, 1::2]).reshape(
        128, 5 * (T // 2)).astype(BF16_NP)
    siE = siAB[:, :, 0::2].reshape(128, 5 * (T // 2)).astype(FP8_NP)
    sp = sip[core * B_PER:(core + 1) * B_PER]         # [4,156,2048]
    # sipT [128, KT*NB2]: [p, k*NB2 + b*CP + c'] = sip[b, c', 128k+p]
    sipT = np.ascontiguousarray(
        sp.transpose(2, 0, 1).reshape(KT, 128, NB2)
        .transpose(1, 0, 2).reshape(128, KT * NB2)
    ).astype(FP8_NP)
    return {"siQ": siQ, "siE": siE, "sipT": sipT}


def prep_shared_inputs(W1, W2, Wl1, Wl2):
    """Weight layouts + threshold tensor, shared by all cores."""
    # W1c [128, 5*512] fp8: region b = tail weights for sample b at rows
    # 32b..32b+27 (zero elsewhere); region 4 = W1^T[0:128].
    w1t = np.zeros((160, HID), dtype=np.float32)
    w1t[:C_IN] = W1.T
    W1c = np.zeros((128, 5 * 512), dtype=FP8_NP)
    W1c[:, 4 * 512:5 * 512] = w1t[:128]
    for b in range(B_PER):
        W1c[32 * b:32 * b + (C_IN - 128), b * 512:(b + 1) * 512] = \
            w1t[128:C_IN]

    # fc2 block weights for DoubleRow: per (kp, b, s) a [128, 128] block,
    # zero except cols 32b..32b+20 = k_scale[m]*W2_m^T (m = 2kp+s). Per-m
    # scale matches the sg encoding: ACT Sign (+-1, m0..m2) -> 0.5x, DVE
    # (+-.5, m3) -> 1.0x. Layout [128, 2*4*2*128].
    # scale per (b, m): DVE is_ge-0.5 encoding (+-.5) -> 1.0x, ACT Sign
    # (+-1) -> 0.5x; routing must match THR_DVE in the device program.
    w2t = W2.T.astype(np.float32)                     # [512, 20]
    W2pT = np.zeros((128, 2 * 4 * 2 * 128), dtype=FP8_NP)
    for kp in range(2):
        for s in range(2):
            m = 2 * kp + s
            for b in range(B_PER):
                sc = 1.0 if (b, m) in THR_DVE else 0.5
                blk = (sc * w2t[m * 128:(m + 1) * 128]).astype(FP8_NP)
                base = ((kp * 4 + b) * 2 + s) * 128 + 32 * b
                W2pT[:, base:base + OUT_DIM] = blk
    # effective (device) W2 after fp8 rounding, unscaled (b=0 blocks)
    w2_eff = np.empty((HID, OUT_DIM), dtype=np.float32)
    for m in range(4):
        kp, s = m // 2, m % 2
        sc = 1.0 if (0, m) in THR_DVE else 0.5
        base = ((kp * 4 + 0) * 2 + s) * 128 + 0
        w2_eff[m * 128:(m + 1) * 128] = (
            W2pT[:, base:base + OUT_DIM].astype(np.float32) / sc)
    r2 = w2_eff.sum(axis=0)                           # [20]
    g = (1.0 - ALPHA ** (np.arange(T, dtype=np.float64) + 1)) / (1.0 - ALPHA)
    T2 = (THETA - 0.5 * np.outer(r2, g)).astype(np.float32)   # [20, T]
    T2 = T2.astype(BF16_NP)

    # Wl1T [128, KT*HID]: [p, k*HID+o] = Wl1[o, 128k+p]
    Wl1T = np.ascontiguousarray(
        Wl1.T.reshape(KT, 128, HID).transpose(1, 0, 2).reshape(128, KT * HID)
    ).astype(FP8_NP)
    # Wl2T [128, 4*32] fp8 (o padded to 32): [p, k*32+o] = Wl2[o, 128k+p]
    wl2t = Wl2.T.reshape(4, 128, OUT_DIM)
    Wl2T = np.zeros((4, 128, 32), dtype=np.float32)
    Wl2T[:, :, :OUT_DIM] = wl2t
    Wl2T = np.ascontiguousarray(
        Wl2T.transpose(1, 0, 2).reshape(128, 4 * 32)).astype(FP8_NP)
    return {"W1c": W1c, "W2pT": W2pT, "Wl1T": Wl1T,
            "Wl2T": Wl2T, "T2": T2}


def make_in_maps(spike_input, W1, W2, Wl1, Wl2, perm):
    si = np.asarray(spike_input, dtype=np.float32).reshape(B, C_IN, T)
    perm = np.asarray(perm).astype(np.int64)
    sip = si[:, perm, :]                              # perm-gather (layout only)
    shared = prep_shared_inputs(np.asarray(W1, np.float32),
                                np.asarray(W2, np.float32),
                                np.asarray(Wl1, np.float32),
                                np.asarray(Wl2, np.float32))
    in_maps = []
    for core in range(N_CORES):
        m = dict(shared)
        m.update(prep_core_inputs(si, sip, core))
        in_maps.append(m)
    return in_maps


_IN_SPECS = {
    "siQ": ((128, 5 * (T // 2)), BF16),
    "siE": ((128, 5 * (T // 2)), FP8),
    "sipT": ((128, KT * NB2), FP8),
    "W1c": ((128, 5 * 512), FP8),
    "W2pT": ((128, 2 * 4 * 2 * 128), FP8),
    "Wl1T": ((128, KT * HID), FP8),
    "Wl2T": ((128, 4 * 32), FP8),
    "T2": ((OUT_DIM, T), BF16),
}


def build_bass():
    nc = bacc.Bacc("TRN2", target_bir_lowering=False, debug=False)
    ins = {}
    for name, (shape, dt) in _IN_SPECS.items():
        h = nc.dram_tensor(name, list(shape), dt, kind="ExternalInput")
        ins[name] = h[:]
    out_h = nc.dram_tensor("out", [B_PER, 32, T + CP], BF16,
                           kind="ExternalOutput")
    outs = {"out": out_h[:]}
    with tile_mod.TileContext(nc) as tc:
        build_program(tc, outs, ins)
    nc.compile()
    return nc


_NC_CACHE = None


def run(inputs, trace=False, **kw):
    """Run on the 8 NeuronCores; returns (full_output, BassKernelResults)."""
    global _NC_CACHE
    if _NC_CACHE is None:
        _NC_CACHE = build_bass()
    nc = _NC_CACHE
    in_maps = make_in_maps(**inputs)
    res = run_bass_kernel_spmd(nc, in_maps, core_ids=list(range(N_CORES)),
                               trace=trace, **kw)
    parts = [res.results[c]["out"][:, :OUT_DIM, :] for c in range(N_CORES)]
    full = np.concatenate(parts, axis=0).reshape(B, OUT_DIM, 1, 1, T + CP)
    return np.ascontiguousarray(full.astype(np.float32)), res


def kernel(**inputs):
    out, _ = run(inputs)
    return out
